# revision 4
# baseline (speedup 1.0000x reference)
"""Trainium2 Bass kernel for masked-attention-like module:
    q = x@Wq; k = x@Wk; v = x@Wv
    scores = (q @ k.T) * tril(l)
    out = scores @ v
T=8192, D_IN=512, D_QK=D_V=64, fp32 inputs/outputs, 8 NeuronCores.

Strategy (sequence-parallel over T, load-balanced over the tril):
  - Work is tiled into [512 t x 512 s] macro-tiles of the lower triangle.
    Core c owns two t-panels: rows [512c, 512c+512) and
    [8192-512(c+1), 8192-512c).  That gives every core exactly 17
    macro-tiles -> identical, branch-free SPMD program.
  - Phase 1 (small SPMD kernel): each core computes qT/kT (fp16) and v
    (fp16) for its own 1024 rows from a host-pre-transposed x block.
  - Host gathers the tiny projections (pure data movement), then packs
    per-core, per-macro-tile operand arrays.  The l tile for each
    macro-tile is pre-masked (tril) and pre-transposed on the host, so
    the device computes scores directly in transposed layout:
        S^T[s,t] = sum_n kT[n,s] qT[n,t]   (PE, fp16)
        Sm^T = S^T * lT                    (DVE, fp32 l, fp16 out)
        outT[i] += v[s-chunk].T-free @ ...  (PE, fp16, accumulate)
    -> no on-chip transposes at all.
  - Phase 2 emits per-macro-tile partial outputs outT [64, 512]; the
    host sums partials per panel and transposes into the final [T, 64].
"""

import json

import numpy as np

T = 8192
D_IN = 512
D_QK = 64
D_V = 64
NCORES = 8
PANEL = 512  # rows per t-panel
NITEMS = 17  # macro-tiles per core

# ---------------------------------------------------------------------------
# Workaround: the walrus build in this container accepts only ONE sync-wait
# per instruction, but Tile attaches several (e.g. to the tail Drain).  Split
# multi-wait instructions at the BIR-JSON level by inserting single-wait NoOps
# on the same engine immediately before the instruction.
# ---------------------------------------------------------------------------
_fix_installed = [False]
_split_counter = [0]


def _fix_bir_json(bir_json):
    m = json.loads(bir_json)
    for f in m.get("functions", []):
        for blk in f.get("blocks", []):
            new_insts = []
            for inst in blk.get("instructions", []):
                si = inst.get("sync_info") or {}
                waits = si.get("on_wait") or []
                if len(waits) > 1:
                    for w in waits[:-1]:
                        _split_counter[0] += 1
                        new_insts.append({
                            "name": f"I-waitsplit-{_split_counter[0]}",
                            "opcode": "NoOp",
                            "engine": inst.get("engine"),
                            "ins": [],
                            "outs": [],
                            "sync_info": {"on_wait": [w], "on_update": []},
                        })
                    si = dict(si)
                    si["on_wait"] = waits[-1:]
                    inst = dict(inst)
                    inst["sync_info"] = si
                new_insts.append(inst)
            blk["instructions"] = new_insts
    return json.dumps(m).encode()


def _install_bir_fix():
    if _fix_installed[0]:
        return
    _fix_installed[0] = True
    import concourse.bass_utils as bu
    import concourse.bass2jax as b2j

    orig = bu.compile_bir_kernel

    def patched(bir_json, tmpdir, neff_name="file.neff"):
        return orig(_fix_bir_json(bir_json), tmpdir, neff_name)

    bu.compile_bir_kernel = patched
    b2j.compile_bir_kernel = patched


# ---------------------------------------------------------------------------
# Per-core work-item list: (t0, s0) macro-tile origins, 17 per core.
# ---------------------------------------------------------------------------
def _core_items(c):
    tA = 512 * c
    tB = T - 512 * (c + 1)
    items = [(tA, 512 * j) for j in range(c + 1)]
    items += [(tB, 512 * j) for j in range(16 - c)]
    assert len(items) == NITEMS
    return items


# ---------------------------------------------------------------------------
# Bass kernel builders
# ---------------------------------------------------------------------------
def _build_phase1():
    import concourse.bass as bass
    import concourse.mybir as mybir
    from concourse.tile import TileContext

    f32 = mybir.dt.float32
    f16 = mybir.dt.float16

    nc = bass.Bass(target_bir_lowering=False, trn_type="TRN2")
    # inputs: host-packed transposed x block + packed weights
    xTp = nc.dram_tensor("xTp", [128, 4 * 1024], f32, kind="ExternalInput")
    Wqp = nc.dram_tensor("Wqp", [128, 4 * 64], f32, kind="ExternalInput")
    Wkp = nc.dram_tensor("Wkp", [128, 4 * 64], f32, kind="ExternalInput")
    Wvp = nc.dram_tensor("Wvp", [128, 4 * 64], f32, kind="ExternalInput")
    qT_o = nc.dram_tensor("qT_o", [64, 1024], f16, kind="ExternalOutput")
    kT_o = nc.dram_tensor("kT_o", [64, 1024], f16, kind="ExternalOutput")
    v_o = nc.dram_tensor("v_o", [128, 8 * 64], f16, kind="ExternalOutput")

    with TileContext(nc) as tc:
        with (
            tc.tile_pool(name="sb", bufs=1) as sb,
            tc.tile_pool(name="outp", bufs=2) as outp,
            tc.tile_pool(name="ps", bufs=2, space="PSUM") as ps,
        ):
            xt = sb.tile([128, 4 * 1024], f32, tag="xt")
            wq = sb.tile([128, 256], f32, tag="wq")
            wk = sb.tile([128, 256], f32, tag="wk")
            wv = sb.tile([128, 256], f32, tag="wv")
            nc.sync.dma_start(xt[:], xTp[:])
            nc.sync.dma_start(wq[:], Wqp[:])
            nc.sync.dma_start(wk[:], Wkp[:])
            nc.sync.dma_start(wv[:], Wvp[:])
            # round operands to fp16 on-device, then run projections in fp16
            xt16 = sb.tile([128, 4 * 1024], f16, tag="xt16")
            wq16 = sb.tile([128, 256], f16, tag="wq16")
            wk16 = sb.tile([128, 256], f16, tag="wk16")
            wv16 = sb.tile([128, 256], f16, tag="wv16")
            nc.vector.tensor_copy(xt16[:], xt[:])
            nc.vector.tensor_copy(wq16[:], wq[:])
            nc.vector.tensor_copy(wk16[:], wk[:])
            nc.vector.tensor_copy(wv16[:], wv[:])

            # qT / kT: [64, 1024] = W.T @ x.T, contracted over d in 4 chunks
            for w16, dst in ((wq16, qT_o), (wk16, kT_o)):
                for sbk in range(2):
                    pq = ps.tile([64, 512], f32, tag="pq")
                    for dc in range(4):
                        nc.tensor.matmul(
                            pq[:],
                            w16[:, dc * 64:(dc + 1) * 64],
                            xt16[:, dc * 1024 + sbk * 512: dc * 1024 + sbk * 512 + 512],
                            start=(dc == 0),
                            stop=(dc == 3),
                        )
                    ot = outp.tile([64, 512], f16, tag="po")
                    nc.vector.tensor_copy(ot[:], pq[:])
                    nc.sync.dma_start(dst[:, sbk * 512:(sbk + 1) * 512], ot[:])
            # v: [128 s, 64] per s-chunk = x.T-chunk.T @ Wv
            for sc in range(8):
                pv = ps.tile([128, 64], f32, tag="pv")
                for dc in range(4):
                    nc.tensor.matmul(
                        pv[:],
                        xt16[:, dc * 1024 + sc * 128: dc * 1024 + sc * 128 + 128],
                        wv16[:, dc * 64:(dc + 1) * 64],
                        start=(dc == 0),
                        stop=(dc == 3),
                    )
                vt = outp.tile([128, 64], f16, tag="pvo")
                nc.vector.tensor_copy(vt[:], pv[:])
                nc.sync.dma_start(v_o[:, sc * 64:(sc + 1) * 64], vt[:])
    return nc


def _build_phase2():
    import concourse.bass as bass
    import concourse.mybir as mybir
    from concourse.tile import TileContext

    f32 = mybir.dt.float32
    f16 = mybir.dt.float16

    nc = bass.Bass(target_bir_lowering=False, trn_type="TRN2")
    lwp = nc.dram_tensor("lwp", [NITEMS, 128, 2048], f32, kind="ExternalInput")
    kTwp = nc.dram_tensor("kTwp", [NITEMS, 64, 512], f16, kind="ExternalInput")
    qTwp = nc.dram_tensor("qTwp", [NITEMS, 64, 512], f16, kind="ExternalInput")
    vwp = nc.dram_tensor("vwp", [NITEMS, 128, 256], f16, kind="ExternalInput")
    po = nc.dram_tensor("po", [NITEMS, 64, 512], f32, kind="ExternalOutput")

    with TileContext(nc) as tc:
        with (
            tc.tile_pool(name="lw", bufs=3) as lwpool,
            tc.tile_pool(name="ops", bufs=3) as ops,
            tc.tile_pool(name="smt", bufs=8) as smtp,
            tc.tile_pool(name="osb", bufs=2) as osb,
            tc.tile_pool(name="psS", bufs=3, space="PSUM") as psS,
            tc.tile_pool(name="psO", bufs=2, space="PSUM") as psO,
        ):
            for i in range(NITEMS):
                lw = lwpool.tile([128, 2048], f32, tag="lw")
                kt = ops.tile([64, 512], f16, tag="kt")
                qt = ops.tile([64, 512], f16, tag="qt")
                vw = ops.tile([128, 256], f16, tag="vw")
                nc.sync.dma_start(lw[:], lwp[i])
                nc.sync.dma_start(kt[:], kTwp[i])
                nc.sync.dma_start(qt[:], qTwp[i])
                nc.sync.dma_start(vw[:], vwp[i])
                out_ps = psO.tile([64, 512], f32, tag="out")
                for sc in range(4):
                    s_ps = psS.tile([128, 512], f32, tag="S")
                    nc.tensor.matmul(
                        s_ps[:],
                        kt[:, sc * 128:(sc + 1) * 128],
                        qt[:],
                        start=True,
                        stop=True,
                    )
                    smt = smtp.tile([128, 512], f16, tag="smt")
                    nc.vector.tensor_mul(
                        smt[:], s_ps[:], lw[:, sc * 512:(sc + 1) * 512]
                    )
                    nc.tensor.matmul(
                        out_ps[:],
                        vw[:, sc * 64:(sc + 1) * 64],
                        smt[:],
                        start=(sc == 0),
                        stop=(sc == 3),
                    )
                ot = osb.tile([64, 512], f32, tag="ot")
                nc.scalar.copy(ot[:], out_ps[:])
                nc.sync.dma_start(po[i], ot[:])
    return nc


_nc_cache = {}


def _get_nc(which):
    if which not in _nc_cache:
        _nc_cache[which] = _build_phase1() if which == 1 else _build_phase2()
    return _nc_cache[which]


# ---------------------------------------------------------------------------
# Host-side packing helpers (pure data movement)
# ---------------------------------------------------------------------------
def _pack_chunks(a, nchunk, rows):
    """[nchunk*rows, w] -> [rows, nchunk*w] with chunk-major free dim."""
    w = a.shape[1]
    return np.ascontiguousarray(
        a.reshape(nchunk, rows, w).transpose(1, 0, 2).reshape(rows, nchunk * w)
    )


def kernel(x, Wq, Wk, Wv, l):
    _install_bir_fix()
    from concourse import bass_utils

    x = np.asarray(x, dtype=np.float32)
    Wq = np.asarray(Wq, dtype=np.float32)
    Wk = np.asarray(Wk, dtype=np.float32)
    Wv = np.asarray(Wv, dtype=np.float32)
    l = np.asarray(l, dtype=np.float32)

    core_ids = list(range(NCORES))

    # ---------------- Phase 1: per-core projections -----------------------
    wq_p = _pack_chunks(Wq, 4, 128)
    wk_p = _pack_chunks(Wk, 4, 128)
    wv_p = _pack_chunks(Wv, 4, 128)
    in1 = []
    panels = []
    for c in range(NCORES):
        tA = 512 * c
        tB = T - 512 * (c + 1)
        panels.append((tA, tB))
        xcat = np.concatenate([x[tA:tA + 512], x[tB:tB + 512]], axis=0)  # [1024, 512]
        xT = np.ascontiguousarray(xcat.T)  # [512, 1024]
        xTp = _pack_chunks(xT, 4, 128)  # [128, 4096]
        in1.append({"xTp": xTp, "Wqp": wq_p, "Wkp": wk_p, "Wvp": wv_p})

    res1 = bass_utils.run_bass_kernel_spmd(_get_nc(1), in1, core_ids=core_ids)

    qT_full = np.empty((64, T), dtype=np.float16)
    kT_full = np.empty((64, T), dtype=np.float16)
    v_full = np.empty((T, 64), dtype=np.float16)
    for c in range(NCORES):
        tA, tB = panels[c]
        r = res1.results[c]
        qT_full[:, tA:tA + 512] = r["qT_o"][:, :512]
        qT_full[:, tB:tB + 512] = r["qT_o"][:, 512:]
        kT_full[:, tA:tA + 512] = r["kT_o"][:, :512]
        kT_full[:, tB:tB + 512] = r["kT_o"][:, 512:]
        vup = r["v_o"].reshape(128, 8, 64).transpose(1, 0, 2).reshape(1024, 64)
        v_full[tA:tA + 512] = vup[:512]
        v_full[tB:tB + 512] = vup[512:]

    # ---------------- Phase 2: masked scores + PV -------------------------
    in2 = []
    for c in range(NCORES):
        items = _core_items(c)
        lwp = np.empty((NITEMS, 128, 2048), dtype=np.float32)
        kTwp = np.empty((NITEMS, 64, 512), dtype=np.float16)
        qTwp = np.empty((NITEMS, 64, 512), dtype=np.float16)
        vwp = np.empty((NITEMS, 128, 256), dtype=np.float16)
        for i, (t0, s0) in enumerate(items):
            lt = l[t0:t0 + 512, s0:s0 + 512]
            if t0 == s0:
                lt = np.tril(lt)
            lT = lt.T  # [512 s, 512 t]
            lwp[i] = lT.reshape(4, 128, 512).transpose(1, 0, 2).reshape(128, 2048)
            kTwp[i] = kT_full[:, s0:s0 + 512]
            qTwp[i] = qT_full[:, t0:t0 + 512]
            vwp[i] = (
                v_full[s0:s0 + 512]
                .reshape(4, 128, 64)
                .transpose(1, 0, 2)
                .reshape(128, 256)
            )
        in2.append({"lwp": lwp, "kTwp": kTwp, "qTwp": qTwp, "vwp": vwp})

    res2 = bass_utils.run_bass_kernel_spmd(_get_nc(2), in2, core_ids=core_ids)

    out = np.empty((T, 64), dtype=np.float32)
    for c in range(NCORES):
        items = _core_items(c)
        tA, tB = panels[c]
        p = res2.results[c]["po"]  # [17, 64, 512]
        nA = c + 1
        out[tA:tA + 512] = p[:nA].sum(axis=0).T
        out[tB:tB + 512] = p[nA:].sum(axis=0).T
    return out


# revision 11
# speedup vs baseline: 1.2112x; 1.2112x over previous
"""Trainium2 Bass kernel for masked-attention-like module:
    q = x@Wq; k = x@Wk; v = x@Wv
    scores = (q @ k.T) * tril(l)
    out = scores @ v
T=8192, D_IN=512, D_QK=D_V=64, fp32 inputs/outputs, 8 NeuronCores.

Strategy (sequence-parallel over T, load-balanced over the tril):
  - Work is tiled into [512 t x 512 s] macro-tiles of the lower triangle.
    Core c owns two t-panels: rows [512c, 512c+512) and
    [8192-512(c+1), 8192-512c).  That gives every core exactly 17
    macro-tiles -> identical, branch-free SPMD program.
  - Phase 1 (small SPMD kernel): each core computes qT/kT (fp16) and v
    (fp16) for its own 1024 rows from a host-pre-transposed x block.
  - Host gathers the tiny projections (pure data movement), then packs
    per-core, per-macro-tile operand arrays.  The l tile for each
    macro-tile is pre-masked (tril) and pre-transposed on the host, so
    the device computes scores directly in transposed layout:
        S^T[s,t] = sum_n kT[n,s] qT[n,t]   (PE, fp16)
        Sm^T = S^T * lT                    (DVE, fp32 l, fp16 out)
        outT[i] += v[s-chunk].T-free @ ...  (PE, fp16, accumulate)
    -> no on-chip transposes at all.
  - Phase 2 emits per-macro-tile partial outputs outT [64, 512]; the
    host sums partials per panel and transposes into the final [T, 64].
"""

import json

import numpy as np

T = 8192
D_IN = 512
D_QK = 64
D_V = 64
NCORES = 8
PANEL = 512  # rows per t-panel
NITEMS = 17  # macro-tiles per core

# ---------------------------------------------------------------------------
# Workaround: the walrus build in this container accepts only ONE sync-wait
# per instruction, but Tile attaches several (e.g. to the tail Drain).  Split
# multi-wait instructions at the BIR-JSON level by inserting single-wait NoOps
# on the same engine immediately before the instruction.
# ---------------------------------------------------------------------------
_fix_installed = [False]
_split_counter = [0]


def _fix_bir_json(bir_json):
    m = json.loads(bir_json)
    for f in m.get("functions", []):
        for blk in f.get("blocks", []):
            new_insts = []
            for inst in blk.get("instructions", []):
                si = inst.get("sync_info") or {}
                waits = si.get("on_wait") or []
                if len(waits) > 1:
                    for w in waits[:-1]:
                        _split_counter[0] += 1
                        new_insts.append({
                            "name": f"I-waitsplit-{_split_counter[0]}",
                            "opcode": "NoOp",
                            "engine": inst.get("engine"),
                            "ins": [],
                            "outs": [],
                            "sync_info": {"on_wait": [w], "on_update": []},
                        })
                    si = dict(si)
                    si["on_wait"] = waits[-1:]
                    inst = dict(inst)
                    inst["sync_info"] = si
                new_insts.append(inst)
            blk["instructions"] = new_insts
    return json.dumps(m).encode()


def _install_bir_fix():
    if _fix_installed[0]:
        return
    _fix_installed[0] = True
    import concourse.bass_utils as bu
    import concourse.bass2jax as b2j

    orig = bu.compile_bir_kernel

    def patched(bir_json, tmpdir, neff_name="file.neff"):
        return orig(_fix_bir_json(bir_json), tmpdir, neff_name)

    bu.compile_bir_kernel = patched
    b2j.compile_bir_kernel = patched


# ---------------------------------------------------------------------------
# Per-core work-item list: (t0, s0) macro-tile origins, 17 per core.
# ---------------------------------------------------------------------------
def _core_items(c):
    tA = 512 * c
    tB = T - 512 * (c + 1)
    items = [(tA, 512 * j) for j in range(c + 1)]
    items += [(tB, 512 * j) for j in range(16 - c)]
    assert len(items) == NITEMS
    return items


# ---------------------------------------------------------------------------
# Bass kernel builders
# ---------------------------------------------------------------------------
def _build_phase1():
    import concourse.bass as bass
    import concourse.mybir as mybir
    from concourse.tile import TileContext

    f32 = mybir.dt.float32
    f16 = mybir.dt.float16

    nc = bass.Bass(target_bir_lowering=False, trn_type="TRN2")
    # inputs: host-packed transposed x block + packed weights
    xTp = nc.dram_tensor("xTp", [128, 4 * 1024], f32, kind="ExternalInput")
    Wqp = nc.dram_tensor("Wqp", [128, 4 * 64], f32, kind="ExternalInput")
    Wkp = nc.dram_tensor("Wkp", [128, 4 * 64], f32, kind="ExternalInput")
    Wvp = nc.dram_tensor("Wvp", [128, 4 * 64], f32, kind="ExternalInput")
    qT_o = nc.dram_tensor("qT_o", [64, 1024], f16, kind="ExternalOutput")
    kT_o = nc.dram_tensor("kT_o", [64, 1024], f16, kind="ExternalOutput")
    v_o = nc.dram_tensor("v_o", [128, 8 * 64], f16, kind="ExternalOutput")

    with TileContext(nc) as tc:
        with (
            tc.tile_pool(name="sb", bufs=1) as sb,
            tc.tile_pool(name="xchunks", bufs=3) as xch,
            tc.tile_pool(name="outp", bufs=3) as outp,
            tc.tile_pool(name="ps", bufs=1, space="PSUM") as ps,
        ):
            wq = sb.tile([128, 256], f32, tag="wq")
            wk = sb.tile([128, 256], f32, tag="wk")
            wv = sb.tile([128, 256], f32, tag="wv")
            nc.scalar.dma_start(wq[:], Wqp[:])
            nc.scalar.dma_start(wk[:], Wkp[:])
            nc.scalar.dma_start(wv[:], Wvp[:])
            wq16 = sb.tile([128, 256], f16, tag="wq16")
            wk16 = sb.tile([128, 256], f16, tag="wk16")
            wv16 = sb.tile([128, 256], f16, tag="wv16")
            nc.vector.tensor_copy(wq16[:], wq[:])
            nc.vector.tensor_copy(wk16[:], wk[:])
            nc.vector.tensor_copy(wv16[:], wv[:])

            # qT/kT accumulators: one PSUM bank per accumulation group.
            pq = [
                ps.tile([64, 512], f32, tag=f"pq{j}", name=f"pq{j}")
                for j in range(4)
            ]

            # d-chunk pipeline: DMA chunk -> fp16 round -> 4 accumulating MMs;
            # all four fp16 chunks stay resident for the v pass below.
            xc16s = []
            for dc in range(4):
                xc = xch.tile([128, 1024], f32, tag="xc")
                nc.sync.dma_start(xc[:], xTp[:, dc * 1024:(dc + 1) * 1024])
                xc16 = xch.tile([128, 1024], f16, tag="xc16", bufs=4)
                nc.vector.tensor_copy(xc16[:], xc[:])
                xc16s.append(xc16)
                j = 0
                for w16 in (wq16, wk16):
                    for sbk in range(2):
                        nc.tensor.matmul(
                            pq[j][:],
                            w16[:, dc * 64:(dc + 1) * 64],
                            xc16[:, sbk * 512: sbk * 512 + 512],
                            start=(dc == 0),
                            stop=(dc == 3),
                        )
                        j += 1
            j = 0
            for dst in (qT_o, kT_o):
                for sbk in range(2):
                    ot = outp.tile([64, 512], f16, tag="po")
                    nc.vector.tensor_copy(ot[:], pq[j][:])
                    nc.sync.dma_start(dst[:, sbk * 512:(sbk + 1) * 512], ot[:])
                    j += 1
            # v: one PSUM group per s-chunk, rotating through 3 banks.
            vt = outp.tile([128, 512], f16, tag="pvo")
            for sc in range(8):
                pvt = ps.tile([128, 64], f32, tag="pv", bufs=3)
                for dc in range(4):
                    nc.tensor.matmul(
                        pvt[:],
                        xc16s[dc][:, sc * 128: sc * 128 + 128],
                        wv16[:, dc * 64:(dc + 1) * 64],
                        start=(dc == 0),
                        stop=(dc == 3),
                    )
                nc.vector.tensor_copy(vt[:, sc * 64:(sc + 1) * 64], pvt[:])
            nc.sync.dma_start(v_o[:], vt[:])
    return nc


def _build_phase2():
    import concourse.bass as bass
    import concourse.mybir as mybir
    from concourse.tile import TileContext

    f32 = mybir.dt.float32
    f16 = mybir.dt.float16

    nc = bass.Bass(target_bir_lowering=False, trn_type="TRN2")
    lwp = nc.dram_tensor("lwp", [NITEMS, 128, 2048], f32, kind="ExternalInput")
    kTwp = nc.dram_tensor("kTwp", [NITEMS, 64, 512], f16, kind="ExternalInput")
    qTwp = nc.dram_tensor("qTwp", [NITEMS, 64, 512], f16, kind="ExternalInput")
    vwp = nc.dram_tensor("vwp", [NITEMS, 128, 256], f16, kind="ExternalInput")
    po = nc.dram_tensor("po", [NITEMS, 64, 512], f16, kind="ExternalOutput")

    with TileContext(nc) as tc:
        with (
            tc.tile_pool(name="lw", bufs=3) as lwpool,
            tc.tile_pool(name="ops", bufs=3) as ops,
            tc.tile_pool(name="smt", bufs=8) as smtp,
            tc.tile_pool(name="osb", bufs=2) as osb,
            tc.tile_pool(name="psS", bufs=3, space="PSUM") as psS,
            tc.tile_pool(name="psO", bufs=2, space="PSUM") as psO,
        ):
            for i in range(NITEMS):
                lw = lwpool.tile([128, 2048], f32, tag="lw")
                kt = ops.tile([64, 512], f16, tag="kt")
                qt = ops.tile([64, 512], f16, tag="qt")
                vw = ops.tile([128, 256], f16, tag="vw")
                nc.sync.dma_start(lw[:], lwp[i])
                nc.scalar.dma_start(kt[:], kTwp[i])
                nc.scalar.dma_start(qt[:], qTwp[i])
                nc.scalar.dma_start(vw[:], vwp[i])
                out_ps = psO.tile([64, 512], f32, tag="out")
                for sc in range(4):
                    s_ps = psS.tile([128, 512], f32, tag="S")
                    nc.tensor.matmul(
                        s_ps[:],
                        kt[:, sc * 128:(sc + 1) * 128],
                        qt[:],
                        start=True,
                        stop=True,
                    )
                    smt = smtp.tile([128, 512], f16, tag="smt")
                    nc.vector.tensor_mul(
                        smt[:], s_ps[:], lw[:, sc * 512:(sc + 1) * 512]
                    )
                    nc.tensor.matmul(
                        out_ps[:],
                        vw[:, sc * 64:(sc + 1) * 64],
                        smt[:],
                        start=(sc == 0),
                        stop=(sc == 3),
                    )
                ot = osb.tile([64, 512], f16, tag="ot")
                nc.scalar.copy(ot[:], out_ps[:])
                nc.scalar.dma_start(po[i], ot[:])
    return nc


_nc_cache = {}


def _get_nc(which):
    if which not in _nc_cache:
        _nc_cache[which] = _build_phase1() if which == 1 else _build_phase2()
    return _nc_cache[which]


# ---------------------------------------------------------------------------
# Host-side packing helpers (pure data movement)
# ---------------------------------------------------------------------------
def _pack_chunks(a, nchunk, rows):
    """[nchunk*rows, w] -> [rows, nchunk*w] with chunk-major free dim."""
    w = a.shape[1]
    return np.ascontiguousarray(
        a.reshape(nchunk, rows, w).transpose(1, 0, 2).reshape(rows, nchunk * w)
    )


def kernel(x, Wq, Wk, Wv, l):
    _install_bir_fix()
    from concourse import bass_utils

    x = np.asarray(x, dtype=np.float32)
    Wq = np.asarray(Wq, dtype=np.float32)
    Wk = np.asarray(Wk, dtype=np.float32)
    Wv = np.asarray(Wv, dtype=np.float32)
    l = np.asarray(l, dtype=np.float32)

    core_ids = list(range(NCORES))

    # ---------------- Phase 1: per-core projections -----------------------
    wq_p = _pack_chunks(Wq, 4, 128)
    wk_p = _pack_chunks(Wk, 4, 128)
    wv_p = _pack_chunks(Wv, 4, 128)
    in1 = []
    panels = []
    for c in range(NCORES):
        tA = 512 * c
        tB = T - 512 * (c + 1)
        panels.append((tA, tB))
        xcat = np.concatenate([x[tA:tA + 512], x[tB:tB + 512]], axis=0)  # [1024, 512]
        xT = np.ascontiguousarray(xcat.T)  # [512, 1024]
        xTp = _pack_chunks(xT, 4, 128)  # [128, 4096]
        in1.append({"xTp": xTp, "Wqp": wq_p, "Wkp": wk_p, "Wvp": wv_p})

    res1 = bass_utils.run_bass_kernel_spmd(_get_nc(1), in1, core_ids=core_ids)

    qT_full = np.empty((64, T), dtype=np.float16)
    kT_full = np.empty((64, T), dtype=np.float16)
    v_full = np.empty((T, 64), dtype=np.float16)
    for c in range(NCORES):
        tA, tB = panels[c]
        r = res1.results[c]
        qT_full[:, tA:tA + 512] = r["qT_o"][:, :512]
        qT_full[:, tB:tB + 512] = r["qT_o"][:, 512:]
        kT_full[:, tA:tA + 512] = r["kT_o"][:, :512]
        kT_full[:, tB:tB + 512] = r["kT_o"][:, 512:]
        vup = r["v_o"].reshape(128, 8, 64).transpose(1, 0, 2).reshape(1024, 64)
        v_full[tA:tA + 512] = vup[:512]
        v_full[tB:tB + 512] = vup[512:]

    # ---------------- Phase 2: masked scores + PV -------------------------
    in2 = []
    for c in range(NCORES):
        items = _core_items(c)
        lwp = np.empty((NITEMS, 128, 2048), dtype=np.float32)
        kTwp = np.empty((NITEMS, 64, 512), dtype=np.float16)
        qTwp = np.empty((NITEMS, 64, 512), dtype=np.float16)
        vwp = np.empty((NITEMS, 128, 256), dtype=np.float16)
        for i, (t0, s0) in enumerate(items):
            lt = l[t0:t0 + 512, s0:s0 + 512]
            if t0 == s0:
                lt = np.tril(lt)
            lT = lt.T  # [512 s, 512 t]
            lwp[i] = lT.reshape(4, 128, 512).transpose(1, 0, 2).reshape(128, 2048)
            kTwp[i] = kT_full[:, s0:s0 + 512]
            qTwp[i] = qT_full[:, t0:t0 + 512]
            vwp[i] = (
                v_full[s0:s0 + 512]
                .reshape(4, 128, 64)
                .transpose(1, 0, 2)
                .reshape(128, 256)
            )
        in2.append({"lwp": lwp, "kTwp": kTwp, "qTwp": qTwp, "vwp": vwp})

    res2 = bass_utils.run_bass_kernel_spmd(_get_nc(2), in2, core_ids=core_ids)

    out = np.empty((T, 64), dtype=np.float32)
    for c in range(NCORES):
        items = _core_items(c)
        tA, tB = panels[c]
        p = res2.results[c]["po"].astype(np.float32)  # [17, 64, 512]
        nA = c + 1
        out[tA:tA + 512] = p[:nA].sum(axis=0).T
        out[tB:tB + 512] = p[nA:].sum(axis=0).T
    return out


# revision 15
# speedup vs baseline: 1.2225x; 1.0093x over previous
"""Trainium2 Bass kernel for masked-attention-like module:
    q = x@Wq; k = x@Wk; v = x@Wv
    scores = (q @ k.T) * tril(l)
    out = scores @ v
T=8192, D_IN=512, D_QK=D_V=64, fp32 inputs/outputs, 8 NeuronCores.

Strategy (sequence-parallel over T, load-balanced over the tril):
  - Work is tiled into [512 t x 512 s] macro-tiles of the lower triangle.
    Core c owns two t-panels: rows [512c, 512c+512) and
    [8192-512(c+1), 8192-512c).  That gives every core exactly 17
    macro-tiles -> identical, branch-free SPMD program.
  - Phase 1 (small SPMD kernel): each core computes qT/kT (fp16) and v
    (fp16) for its own 1024 rows from a host-pre-transposed x block.
  - Host gathers the tiny projections (pure data movement), then packs
    per-core, per-macro-tile operand arrays.  The l tile for each
    macro-tile is pre-masked (tril) and pre-transposed on the host, so
    the device computes scores directly in transposed layout:
        S^T[s,t] = sum_n kT[n,s] qT[n,t]   (PE, fp16)
        Sm^T = S^T * lT                    (DVE, fp32 l, fp16 out)
        outT[i] += v[s-chunk].T-free @ ...  (PE, fp16, accumulate)
    -> no on-chip transposes at all.
  - Phase 2 emits per-macro-tile partial outputs outT [64, 512]; the
    host sums partials per panel and transposes into the final [T, 64].
"""

import json

import numpy as np

T = 8192
D_IN = 512
D_QK = 64
D_V = 64
NCORES = 8
PANEL = 512  # rows per t-panel
NITEMS = 17  # macro-tiles per core

# ---------------------------------------------------------------------------
# Workaround: the walrus build in this container accepts only ONE sync-wait
# per instruction, but Tile attaches several (e.g. to the tail Drain).  Split
# multi-wait instructions at the BIR-JSON level by inserting single-wait NoOps
# on the same engine immediately before the instruction.
# ---------------------------------------------------------------------------
_fix_installed = [False]
_split_counter = [0]


def _fix_bir_json(bir_json):
    m = json.loads(bir_json)
    for f in m.get("functions", []):
        for blk in f.get("blocks", []):
            new_insts = []
            for inst in blk.get("instructions", []):
                si = inst.get("sync_info") or {}
                waits = si.get("on_wait") or []
                if len(waits) > 1:
                    for w in waits[:-1]:
                        _split_counter[0] += 1
                        new_insts.append({
                            "name": f"I-waitsplit-{_split_counter[0]}",
                            "opcode": "NoOp",
                            "engine": inst.get("engine"),
                            "ins": [],
                            "outs": [],
                            "sync_info": {"on_wait": [w], "on_update": []},
                        })
                    si = dict(si)
                    si["on_wait"] = waits[-1:]
                    inst = dict(inst)
                    inst["sync_info"] = si
                new_insts.append(inst)
            blk["instructions"] = new_insts
    return json.dumps(m).encode()


def _install_bir_fix():
    if _fix_installed[0]:
        return
    _fix_installed[0] = True
    import concourse.bass_utils as bu
    import concourse.bass2jax as b2j

    orig = bu.compile_bir_kernel

    def patched(bir_json, tmpdir, neff_name="file.neff"):
        return orig(_fix_bir_json(bir_json), tmpdir, neff_name)

    bu.compile_bir_kernel = patched
    b2j.compile_bir_kernel = patched


# ---------------------------------------------------------------------------
# Per-core work-item list: (t0, s0) macro-tile origins, 17 per core.
# ---------------------------------------------------------------------------
def _core_items(c):
    """17 macro-tiles: positions 0/1 are the two diagonal tiles (uniform
    across cores), positions 2..16 the fifteen strictly-lower full tiles."""
    tA = 512 * c
    tB = T - 512 * (c + 1)
    items = [(tA, tA), (tB, tB)]
    items += [(tA, 512 * j) for j in range(c)]
    items += [(tB, 512 * j) for j in range(15 - c)]
    assert len(items) == NITEMS
    return items


# ---------------------------------------------------------------------------
# Bass kernel builders
# ---------------------------------------------------------------------------
def _build_phase1():
    import concourse.bass as bass
    import concourse.mybir as mybir
    from concourse.tile import TileContext

    f32 = mybir.dt.float32
    f16 = mybir.dt.float16

    nc = bass.Bass(target_bir_lowering=False, trn_type="TRN2")
    # inputs: host-packed transposed x block + packed weights
    xTp = nc.dram_tensor("xTp", [128, 4 * 1024], f32, kind="ExternalInput")
    Wqp = nc.dram_tensor("Wqp", [128, 4 * 64], f32, kind="ExternalInput")
    Wkp = nc.dram_tensor("Wkp", [128, 4 * 64], f32, kind="ExternalInput")
    Wvp = nc.dram_tensor("Wvp", [128, 4 * 64], f32, kind="ExternalInput")
    qT_o = nc.dram_tensor("qT_o", [64, 1024], f16, kind="ExternalOutput")
    kT_o = nc.dram_tensor("kT_o", [64, 1024], f16, kind="ExternalOutput")
    v_o = nc.dram_tensor("v_o", [128, 8 * 64], f16, kind="ExternalOutput")

    with TileContext(nc) as tc:
        with (
            tc.tile_pool(name="sb", bufs=1) as sb,
            tc.tile_pool(name="xchunks", bufs=3) as xch,
            tc.tile_pool(name="outp", bufs=3) as outp,
            tc.tile_pool(name="ps", bufs=1, space="PSUM") as ps,
        ):
            wq = sb.tile([128, 256], f32, tag="wq")
            wk = sb.tile([128, 256], f32, tag="wk")
            wv = sb.tile([128, 256], f32, tag="wv")
            nc.scalar.dma_start(wq[:], Wqp[:])
            nc.scalar.dma_start(wk[:], Wkp[:])
            nc.scalar.dma_start(wv[:], Wvp[:])
            wq16 = sb.tile([128, 256], f16, tag="wq16")
            wk16 = sb.tile([128, 256], f16, tag="wk16")
            wv16 = sb.tile([128, 256], f16, tag="wv16")
            nc.vector.tensor_copy(wq16[:], wq[:])
            nc.vector.tensor_copy(wk16[:], wk[:])
            nc.vector.tensor_copy(wv16[:], wv[:])

            # qT/kT accumulators: one PSUM bank per accumulation group.
            pq = [
                ps.tile([64, 512], f32, tag=f"pq{j}", name=f"pq{j}")
                for j in range(4)
            ]

            # d-chunk pipeline: DMA chunk -> fp16 round -> 4 accumulating MMs;
            # all four fp16 chunks stay resident for the v pass below.
            xc16s = []
            for dc in range(4):
                xc = xch.tile([128, 1024], f32, tag="xc")
                nc.sync.dma_start(xc[:], xTp[:, dc * 1024:(dc + 1) * 1024])
                xc16 = xch.tile([128, 1024], f16, tag="xc16", bufs=4)
                nc.vector.tensor_copy(xc16[:], xc[:])
                xc16s.append(xc16)
                j = 0
                for w16 in (wq16, wk16):
                    for sbk in range(2):
                        nc.tensor.matmul(
                            pq[j][:],
                            w16[:, dc * 64:(dc + 1) * 64],
                            xc16[:, sbk * 512: sbk * 512 + 512],
                            start=(dc == 0),
                            stop=(dc == 3),
                        )
                        j += 1
            j = 0
            for dst in (qT_o, kT_o):
                for sbk in range(2):
                    ot = outp.tile([64, 512], f16, tag="po")
                    nc.vector.tensor_copy(ot[:], pq[j][:])
                    nc.sync.dma_start(dst[:, sbk * 512:(sbk + 1) * 512], ot[:])
                    j += 1
            # v: one PSUM group per s-chunk, rotating through 3 banks.
            vt = outp.tile([128, 512], f16, tag="pvo")
            for sc in range(8):
                pvt = ps.tile([128, 64], f32, tag="pv", bufs=3)
                for dc in range(4):
                    nc.tensor.matmul(
                        pvt[:],
                        xc16s[dc][:, sc * 128: sc * 128 + 128],
                        wv16[:, dc * 64:(dc + 1) * 64],
                        start=(dc == 0),
                        stop=(dc == 3),
                    )
                nc.vector.tensor_copy(vt[:, sc * 64:(sc + 1) * 64], pvt[:])
            nc.sync.dma_start(v_o[:], vt[:])
    return nc


def _build_phase2():
    import concourse.bass as bass
    import concourse.mybir as mybir
    from concourse.tile import TileContext

    f32 = mybir.dt.float32
    f16 = mybir.dt.float16

    nc = bass.Bass(target_bir_lowering=False, trn_type="TRN2")
    # items 0/1 are the diagonal tiles (dense-packed lower-tri chunks only),
    # items 2..16 the full lower tiles.
    lwd = nc.dram_tensor("lwd", [2, 128, 1280], f32, kind="ExternalInput")
    lwp = nc.dram_tensor("lwp", [NITEMS - 2, 128, 2048], f32, kind="ExternalInput")
    kTwp = nc.dram_tensor("kTwp", [NITEMS, 64, 512], f16, kind="ExternalInput")
    qTwp = nc.dram_tensor("qTwp", [NITEMS, 64, 512], f16, kind="ExternalInput")
    vwp = nc.dram_tensor("vwp", [NITEMS, 128, 256], f16, kind="ExternalInput")
    po = nc.dram_tensor("po", [NITEMS, 64, 512], f16, kind="ExternalOutput")

    DIAG_OFF = [0, 512, 896, 1152]  # prefix sums of widths 512,384,256,128

    with TileContext(nc) as tc:
        with (
            tc.tile_pool(name="lw", bufs=4) as lwpool,
            tc.tile_pool(name="ops", bufs=4) as ops,
            tc.tile_pool(name="smt", bufs=8) as smtp,
            tc.tile_pool(name="osb", bufs=3) as osb,
            tc.tile_pool(name="psS", bufs=4, space="PSUM") as psS,
            tc.tile_pool(name="psO", bufs=2, space="PSUM") as psO,
        ):
            for i in range(NITEMS):
                diag = i < 2
                if diag:
                    lw = lwpool.tile([128, 1280], f32, tag="lwd", bufs=2)
                    nc.sync.dma_start(lw[:], lwd[i])
                else:
                    lw = lwpool.tile([128, 2048], f32, tag="lw")
                    nc.sync.dma_start(lw[:], lwp[i - 2])
                kt = ops.tile([64, 512], f16, tag="kt")
                qt = ops.tile([64, 512], f16, tag="qt")
                vw = ops.tile([128, 256], f16, tag="vw")
                nc.scalar.dma_start(kt[:], kTwp[i])
                nc.scalar.dma_start(qt[:], qTwp[i])
                nc.scalar.dma_start(vw[:], vwp[i])
                out_ps = psO.tile([64, 512], f32, tag="out")
                for sc in range(4):
                    if diag:
                        w = 512 - 128 * sc
                        t0, off = 128 * sc, DIAG_OFF[sc]
                    else:
                        w, t0, off = 512, 0, 512 * sc
                    s_ps = psS.tile([128, 512], f32, tag="S")
                    nc.tensor.matmul(
                        s_ps[:, :w],
                        kt[:, sc * 128:(sc + 1) * 128],
                        qt[:, t0:512],
                        start=True,
                        stop=True,
                    )
                    smt = smtp.tile([128, 512], f16, tag="smt")
                    nc.vector.tensor_mul(
                        smt[:, :w], s_ps[:, :w], lw[:, off:off + w]
                    )
                    nc.tensor.matmul(
                        out_ps[:, t0:512],
                        vw[:, sc * 64:(sc + 1) * 64],
                        smt[:, :w],
                        start=(sc == 0),
                        stop=(sc == 3),
                    )
                ot = osb.tile([64, 512], f16, tag="ot")
                nc.scalar.copy(ot[:], out_ps[:])
                nc.scalar.dma_start(po[i], ot[:])
    return nc


_nc_cache = {}


def _get_nc(which):
    if which not in _nc_cache:
        _nc_cache[which] = _build_phase1() if which == 1 else _build_phase2()
    return _nc_cache[which]


# ---------------------------------------------------------------------------
# Host-side packing helpers (pure data movement)
# ---------------------------------------------------------------------------
def _pack_chunks(a, nchunk, rows):
    """[nchunk*rows, w] -> [rows, nchunk*w] with chunk-major free dim."""
    w = a.shape[1]
    return np.ascontiguousarray(
        a.reshape(nchunk, rows, w).transpose(1, 0, 2).reshape(rows, nchunk * w)
    )


def kernel(x, Wq, Wk, Wv, l):
    _install_bir_fix()
    from concourse import bass_utils

    x = np.asarray(x, dtype=np.float32)
    Wq = np.asarray(Wq, dtype=np.float32)
    Wk = np.asarray(Wk, dtype=np.float32)
    Wv = np.asarray(Wv, dtype=np.float32)
    l = np.asarray(l, dtype=np.float32)

    core_ids = list(range(NCORES))

    # ---------------- Phase 1: per-core projections -----------------------
    wq_p = _pack_chunks(Wq, 4, 128)
    wk_p = _pack_chunks(Wk, 4, 128)
    wv_p = _pack_chunks(Wv, 4, 128)
    in1 = []
    panels = []
    for c in range(NCORES):
        tA = 512 * c
        tB = T - 512 * (c + 1)
        panels.append((tA, tB))
        xcat = np.concatenate([x[tA:tA + 512], x[tB:tB + 512]], axis=0)  # [1024, 512]
        xT = np.ascontiguousarray(xcat.T)  # [512, 1024]
        xTp = _pack_chunks(xT, 4, 128)  # [128, 4096]
        in1.append({"xTp": xTp, "Wqp": wq_p, "Wkp": wk_p, "Wvp": wv_p})

    res1 = bass_utils.run_bass_kernel_spmd(_get_nc(1), in1, core_ids=core_ids)

    qT_full = np.empty((64, T), dtype=np.float16)
    kT_full = np.empty((64, T), dtype=np.float16)
    v_full = np.empty((T, 64), dtype=np.float16)
    for c in range(NCORES):
        tA, tB = panels[c]
        r = res1.results[c]
        qT_full[:, tA:tA + 512] = r["qT_o"][:, :512]
        qT_full[:, tB:tB + 512] = r["qT_o"][:, 512:]
        kT_full[:, tA:tA + 512] = r["kT_o"][:, :512]
        kT_full[:, tB:tB + 512] = r["kT_o"][:, 512:]
        vup = r["v_o"].reshape(128, 8, 64).transpose(1, 0, 2).reshape(1024, 64)
        v_full[tA:tA + 512] = vup[:512]
        v_full[tB:tB + 512] = vup[512:]

    # ---------------- Phase 2: masked scores + PV -------------------------
    in2 = []
    diag_off = [0, 512, 896, 1152]
    for c in range(NCORES):
        items = _core_items(c)
        lwd = np.empty((2, 128, 1280), dtype=np.float32)
        lwp = np.empty((NITEMS - 2, 128, 2048), dtype=np.float32)
        kTwp = np.empty((NITEMS, 64, 512), dtype=np.float16)
        qTwp = np.empty((NITEMS, 64, 512), dtype=np.float16)
        vwp = np.empty((NITEMS, 128, 256), dtype=np.float16)
        for i, (t0, s0) in enumerate(items):
            lt = l[t0:t0 + 512, s0:s0 + 512]
            if i < 2:
                lT = np.tril(lt).T  # [512 s, 512 t], upper-tri in (s,t)
                for sc in range(4):
                    w = 512 - 128 * sc
                    lwd[i][:, diag_off[sc]:diag_off[sc] + w] = (
                        lT[128 * sc:128 * (sc + 1), 128 * sc:512]
                    )
            else:
                lT = lt.T  # [512 s, 512 t]
                lwp[i - 2] = (
                    lT.reshape(4, 128, 512).transpose(1, 0, 2).reshape(128, 2048)
                )
            kTwp[i] = kT_full[:, s0:s0 + 512]
            qTwp[i] = qT_full[:, t0:t0 + 512]
            vwp[i] = (
                v_full[s0:s0 + 512]
                .reshape(4, 128, 64)
                .transpose(1, 0, 2)
                .reshape(128, 256)
            )
        in2.append({"lwd": lwd, "lwp": lwp, "kTwp": kTwp, "qTwp": qTwp, "vwp": vwp})

    res2 = bass_utils.run_bass_kernel_spmd(_get_nc(2), in2, core_ids=core_ids)

    out = np.empty((T, 64), dtype=np.float32)
    for c in range(NCORES):
        items = _core_items(c)
        tA, tB = panels[c]
        p = res2.results[c]["po"].astype(np.float32)  # [17, 64, 512]
        # item 0 = diag A, item 1 = diag B, 2..2+c-1 full A, rest full B
        pa = p[0] + p[2:2 + c].sum(axis=0)
        pb = p[1] + p[2 + c:].sum(axis=0)
        out[tA:tA + 512] = pa.T
        out[tB:tB + 512] = pb.T
    return out


# revision 23
# speedup vs baseline: 1.3250x; 1.0839x over previous
"""Trainium2 Bass kernel for masked-attention-like module:
    q = x@Wq; k = x@Wk; v = x@Wv
    scores = (q @ k.T) * tril(l)
    out = scores @ v
T=8192, D_IN=512, D_QK=D_V=64, fp32 inputs/outputs, 8 NeuronCores.

Strategy (sequence-parallel over T, load-balanced over the tril):
  - Work is tiled into [512 t x 512 s] macro-tiles of the lower triangle.
    Core c owns two t-panels: rows [512c, 512c+512) and
    [8192-512(c+1), 8192-512c).  That gives every core exactly 17
    macro-tiles -> identical, branch-free SPMD program.
  - Phase 1 (small SPMD kernel): each core computes qT/kT (fp16) and v
    (fp16) for its own 1024 rows from a host-pre-transposed x block.
  - Host gathers the tiny projections (pure data movement), then packs
    per-core, per-macro-tile operand arrays.  The l tile for each
    macro-tile is pre-masked (tril) and pre-transposed on the host, so
    the device computes scores directly in transposed layout:
        S^T[s,t] = sum_n kT[n,s] qT[n,t]   (PE, fp16)
        Sm^T = S^T * lT                    (DVE, fp32 l, fp16 out)
        outT[i] += v[s-chunk].T-free @ ...  (PE, fp16, accumulate)
    -> no on-chip transposes at all.
  - Phase 2 emits per-macro-tile partial outputs outT [64, 512]; the
    host sums partials per panel and transposes into the final [T, 64].
"""

import json

import numpy as np

T = 8192
D_IN = 512
D_QK = 64
D_V = 64
NCORES = 8
PANEL = 512  # rows per t-panel
NITEMS = 17  # macro-tiles per core

# ---------------------------------------------------------------------------
# Workaround: the walrus build in this container accepts only ONE sync-wait
# per instruction, but Tile attaches several (e.g. to the tail Drain).  Split
# multi-wait instructions at the BIR-JSON level by inserting single-wait NoOps
# on the same engine immediately before the instruction.
# ---------------------------------------------------------------------------
_fix_installed = [False]
_split_counter = [0]


def _fix_bir_json(bir_json):
    m = json.loads(bir_json)
    for f in m.get("functions", []):
        for blk in f.get("blocks", []):
            new_insts = []
            for inst in blk.get("instructions", []):
                si = inst.get("sync_info") or {}
                waits = si.get("on_wait") or []
                if len(waits) > 1:
                    for w in waits[:-1]:
                        _split_counter[0] += 1
                        new_insts.append({
                            "name": f"I-waitsplit-{_split_counter[0]}",
                            "opcode": "NoOp",
                            "engine": inst.get("engine"),
                            "ins": [],
                            "outs": [],
                            "sync_info": {"on_wait": [w], "on_update": []},
                        })
                    si = dict(si)
                    si["on_wait"] = waits[-1:]
                    inst = dict(inst)
                    inst["sync_info"] = si
                new_insts.append(inst)
            blk["instructions"] = new_insts
    return json.dumps(m).encode()


def _install_bir_fix():
    if _fix_installed[0]:
        return
    _fix_installed[0] = True
    import concourse.bass_utils as bu
    import concourse.bass2jax as b2j

    orig = bu.compile_bir_kernel

    def patched(bir_json, tmpdir, neff_name="file.neff"):
        return orig(_fix_bir_json(bir_json), tmpdir, neff_name)

    bu.compile_bir_kernel = patched
    b2j.compile_bir_kernel = patched


# ---------------------------------------------------------------------------
# Per-core work-item list: (t0, s0) macro-tile origins, 17 per core.
# ---------------------------------------------------------------------------
def _core_items(c):
    """17 macro-tiles: positions 0/1 are the two diagonal tiles (uniform
    across cores), positions 2..16 the fifteen strictly-lower full tiles."""
    tA = 512 * c
    tB = T - 512 * (c + 1)
    items = [(tA, tA), (tB, tB)]
    items += [(tA, 512 * j) for j in range(c)]
    items += [(tB, 512 * j) for j in range(15 - c)]
    assert len(items) == NITEMS
    return items


# ---------------------------------------------------------------------------
# Bass kernel builders
# ---------------------------------------------------------------------------
def _build_phase1():
    import concourse.bass as bass
    import concourse.mybir as mybir
    from concourse.tile import TileContext

    f32 = mybir.dt.float32
    f16 = mybir.dt.float16

    nc = bass.Bass(target_bir_lowering=False, trn_type="TRN2")
    # inputs: host-packed transposed x block + packed weights
    xTp = nc.dram_tensor("xTp", [128, 4 * 1024], f32, kind="ExternalInput")
    Wqp = nc.dram_tensor("Wqp", [128, 4 * 64], f32, kind="ExternalInput")
    Wkp = nc.dram_tensor("Wkp", [128, 4 * 64], f32, kind="ExternalInput")
    Wvp = nc.dram_tensor("Wvp", [128, 4 * 64], f32, kind="ExternalInput")
    qT_o = nc.dram_tensor("qT_o", [64, 1024], f16, kind="ExternalOutput")
    kT_o = nc.dram_tensor("kT_o", [64, 1024], f16, kind="ExternalOutput")
    v_o = nc.dram_tensor("v_o", [128, 8 * 64], f16, kind="ExternalOutput")

    with TileContext(nc) as tc:
        with (
            tc.tile_pool(name="sb", bufs=1) as sb,
            tc.tile_pool(name="xchunks", bufs=3) as xch,
            tc.tile_pool(name="outp", bufs=3) as outp,
            tc.tile_pool(name="ps", bufs=1, space="PSUM") as ps,
        ):
            wq = sb.tile([128, 256], f32, tag="wq")
            wk = sb.tile([128, 256], f32, tag="wk")
            wv = sb.tile([128, 256], f32, tag="wv")
            nc.scalar.dma_start(wq[:], Wqp[:])
            nc.scalar.dma_start(wk[:], Wkp[:])
            nc.scalar.dma_start(wv[:], Wvp[:])
            wq16 = sb.tile([128, 256], f16, tag="wq16")
            wk16 = sb.tile([128, 256], f16, tag="wk16")
            wv16 = sb.tile([128, 256], f16, tag="wv16")
            nc.vector.tensor_copy(wq16[:], wq[:])
            nc.vector.tensor_copy(wk16[:], wk[:])
            nc.vector.tensor_copy(wv16[:], wv[:])

            # qT/kT accumulators: one PSUM bank per accumulation group.
            pq = [
                ps.tile([64, 512], f32, tag=f"pq{j}", name=f"pq{j}")
                for j in range(4)
            ]

            # d-chunk pipeline: DMA chunk -> fp16 round -> 4 accumulating MMs;
            # all four fp16 chunks stay resident for the v pass below.
            xc16s = []
            for dc in range(4):
                xc = xch.tile([128, 1024], f32, tag="xc")
                nc.sync.dma_start(xc[:], xTp[:, dc * 1024:(dc + 1) * 1024])
                xc16 = xch.tile([128, 1024], f16, tag="xc16", bufs=4)
                nc.vector.tensor_copy(xc16[:], xc[:])
                xc16s.append(xc16)
                j = 0
                for w16 in (wq16, wk16):
                    for sbk in range(2):
                        nc.tensor.matmul(
                            pq[j][:],
                            w16[:, dc * 64:(dc + 1) * 64],
                            xc16[:, sbk * 512: sbk * 512 + 512],
                            start=(dc == 0),
                            stop=(dc == 3),
                        )
                        j += 1
            j = 0
            for dst in (qT_o, kT_o):
                for sbk in range(2):
                    ot = outp.tile([64, 512], f16, tag="po")
                    nc.scalar.copy(ot[:], pq[j][:])
                    nc.sync.dma_start(dst[:, sbk * 512:(sbk + 1) * 512], ot[:])
                    j += 1
            # v: one PSUM group per s-chunk, rotating through 3 banks.
            vt = outp.tile([128, 512], f16, tag="pvo")
            for sc in range(8):
                pvt = ps.tile([128, 64], f32, tag="pv", bufs=3)
                for dc in range(4):
                    nc.tensor.matmul(
                        pvt[:],
                        xc16s[dc][:, sc * 128: sc * 128 + 128],
                        wv16[:, dc * 64:(dc + 1) * 64],
                        start=(dc == 0),
                        stop=(dc == 3),
                    )
                nc.vector.tensor_copy(vt[:, sc * 64:(sc + 1) * 64], pvt[:])
            nc.sync.dma_start(v_o[:], vt[:])
    return nc


def _build_phase2():
    import concourse.bass as bass
    import concourse.mybir as mybir
    from concourse.tile import TileContext

    f32 = mybir.dt.float32
    f16 = mybir.dt.float16

    nc = bass.Bass(target_bir_lowering=False, trn_type="TRN2")
    # items 0/1 are the diagonal tiles (dense-packed lower-tri chunks only),
    # items 2..16 the full lower tiles.
    lwd = nc.dram_tensor("lwd", [2, 128, 1280], f32, kind="ExternalInput")
    lwp = nc.dram_tensor("lwp", [NITEMS - 2, 128, 2048], f32, kind="ExternalInput")
    kqwp = nc.dram_tensor("kqwp", [NITEMS, 64, 1024], f16, kind="ExternalInput")
    vwp = nc.dram_tensor("vwp", [NITEMS, 128, 256], f16, kind="ExternalInput")
    po = nc.dram_tensor("po", [NITEMS, 64, 512], f16, kind="ExternalOutput")

    DIAG_OFF = [0, 512, 896, 1152]  # prefix sums of widths 512,384,256,128

    with TileContext(nc) as tc:
        with (
            tc.tile_pool(name="lw", bufs=4) as lwpool,
            tc.tile_pool(name="ops", bufs=4) as ops,
            tc.tile_pool(name="smt", bufs=8) as smtp,
            tc.tile_pool(name="osb", bufs=3) as osb,
            tc.tile_pool(name="psS", bufs=4, space="PSUM") as psS,
            tc.tile_pool(name="psO", bufs=2, space="PSUM") as psO,
        ):
            for i in range(NITEMS):
                diag = i < 2
                if diag:
                    lw = lwpool.tile([128, 1280], f32, tag="lwd", bufs=2)
                    nc.sync.dma_start(lw[:], lwd[i])
                else:
                    lw = lwpool.tile([128, 2048], f32, tag="lw")
                    nc.sync.dma_start(lw[:], lwp[i - 2])
                kq = ops.tile([64, 1024], f16, tag="kq")
                vw = ops.tile([128, 256], f16, tag="vw")
                nc.scalar.dma_start(kq[:], kqwp[i])
                nc.scalar.dma_start(vw[:], vwp[i])
                out_ps = psO.tile([64, 512], f32, tag="out")
                for sc in range(4):
                    if diag:
                        w = 512 - 128 * sc
                        t0, off = 128 * sc, DIAG_OFF[sc]
                    else:
                        w, t0, off = 512, 0, 512 * sc
                    s_ps = psS.tile([128, 512], f32, tag="S")
                    nc.tensor.matmul(
                        s_ps[:, :w],
                        kq[:, sc * 128:(sc + 1) * 128],
                        kq[:, 512 + t0:1024],
                        start=True,
                        stop=True,
                    )
                    smt = smtp.tile([128, 512], f16, tag="smt")
                    nc.vector.tensor_mul(
                        smt[:, :w], s_ps[:, :w], lw[:, off:off + w]
                    )
                    nc.tensor.matmul(
                        out_ps[:, t0:512],
                        vw[:, sc * 64:(sc + 1) * 64],
                        smt[:, :w],
                        start=(sc == 0),
                        stop=(sc == 3),
                    )
                ot = osb.tile([64, 512], f16, tag="ot")
                nc.scalar.copy(ot[:], out_ps[:])
                nc.scalar.dma_start(po[i], ot[:])
    return nc


_nc_cache = {}


def _get_nc(which):
    if which not in _nc_cache:
        _nc_cache[which] = _build_phase1() if which == 1 else _build_phase2()
    return _nc_cache[which]


# ---------------------------------------------------------------------------
# Host-side packing helpers (pure data movement)
# ---------------------------------------------------------------------------
def _pack_chunks(a, nchunk, rows):
    """[nchunk*rows, w] -> [rows, nchunk*w] with chunk-major free dim."""
    w = a.shape[1]
    return np.ascontiguousarray(
        a.reshape(nchunk, rows, w).transpose(1, 0, 2).reshape(rows, nchunk * w)
    )


def kernel(x, Wq, Wk, Wv, l):
    _install_bir_fix()
    from concourse import bass_utils

    x = np.asarray(x, dtype=np.float32)
    Wq = np.asarray(Wq, dtype=np.float32)
    Wk = np.asarray(Wk, dtype=np.float32)
    Wv = np.asarray(Wv, dtype=np.float32)
    l = np.asarray(l, dtype=np.float32)

    core_ids = list(range(NCORES))

    # ---------------- Phase 1: per-core projections -----------------------
    wq_p = _pack_chunks(Wq, 4, 128)
    wk_p = _pack_chunks(Wk, 4, 128)
    wv_p = _pack_chunks(Wv, 4, 128)
    in1 = []
    panels = []
    for c in range(NCORES):
        tA = 512 * c
        tB = T - 512 * (c + 1)
        panels.append((tA, tB))
        xcat = np.concatenate([x[tA:tA + 512], x[tB:tB + 512]], axis=0)  # [1024, 512]
        xT = np.ascontiguousarray(xcat.T)  # [512, 1024]
        xTp = _pack_chunks(xT, 4, 128)  # [128, 4096]
        in1.append({"xTp": xTp, "Wqp": wq_p, "Wkp": wk_p, "Wvp": wv_p})

    res1 = bass_utils.run_bass_kernel_spmd(_get_nc(1), in1, core_ids=core_ids)

    qT_full = np.empty((64, T), dtype=np.float16)
    kT_full = np.empty((64, T), dtype=np.float16)
    v_full = np.empty((T, 64), dtype=np.float16)
    for c in range(NCORES):
        tA, tB = panels[c]
        r = res1.results[c]
        qT_full[:, tA:tA + 512] = r["qT_o"][:, :512]
        qT_full[:, tB:tB + 512] = r["qT_o"][:, 512:]
        kT_full[:, tA:tA + 512] = r["kT_o"][:, :512]
        kT_full[:, tB:tB + 512] = r["kT_o"][:, 512:]
        vup = r["v_o"].reshape(128, 8, 64).transpose(1, 0, 2).reshape(1024, 64)
        v_full[tA:tA + 512] = vup[:512]
        v_full[tB:tB + 512] = vup[512:]

    # ---------------- Phase 2: masked scores + PV -------------------------
    in2 = []
    diag_off = [0, 512, 896, 1152]
    for c in range(NCORES):
        items = _core_items(c)
        lwd = np.empty((2, 128, 1280), dtype=np.float32)
        lwp = np.empty((NITEMS - 2, 128, 2048), dtype=np.float32)
        kqwp = np.empty((NITEMS, 64, 1024), dtype=np.float16)
        vwp = np.empty((NITEMS, 128, 256), dtype=np.float16)
        for i, (t0, s0) in enumerate(items):
            lt = l[t0:t0 + 512, s0:s0 + 512]
            if i < 2:
                lT = np.tril(lt).T  # [512 s, 512 t], upper-tri in (s,t)
                for sc in range(4):
                    w = 512 - 128 * sc
                    lwd[i][:, diag_off[sc]:diag_off[sc] + w] = (
                        lT[128 * sc:128 * (sc + 1), 128 * sc:512]
                    )
            else:
                lT = lt.T  # [512 s, 512 t]
                lwp[i - 2] = (
                    lT.reshape(4, 128, 512).transpose(1, 0, 2).reshape(128, 2048)
                )
            kqwp[i, :, :512] = kT_full[:, s0:s0 + 512]
            kqwp[i, :, 512:] = qT_full[:, t0:t0 + 512]
            vwp[i] = (
                v_full[s0:s0 + 512]
                .reshape(4, 128, 64)
                .transpose(1, 0, 2)
                .reshape(128, 256)
            )
        in2.append({"lwd": lwd, "lwp": lwp, "kqwp": kqwp, "vwp": vwp})

    res2 = bass_utils.run_bass_kernel_spmd(_get_nc(2), in2, core_ids=core_ids)

    out = np.empty((T, 64), dtype=np.float32)
    for c in range(NCORES):
        items = _core_items(c)
        tA, tB = panels[c]
        p = res2.results[c]["po"].astype(np.float32)  # [17, 64, 512]
        # item 0 = diag A, item 1 = diag B, 2..2+c-1 full A, rest full B
        pa = p[0] + p[2:2 + c].sum(axis=0)
        pb = p[1] + p[2 + c:].sum(axis=0)
        out[tA:tA + 512] = pa.T
        out[tB:tB + 512] = pb.T
    return out
